# revision 1
# baseline (speedup 1.0000x reference)
"""CCA few-shot scoring kernel for Trainium2 (8 NeuronCores, SPMD).

Inputs (full): spt [1,5,3840,5,5] f32, qry [75,3840,5,5] f32.
Output: sim [75,5] f32.

Sharding: data-parallel over queries. 75 queries padded to 80; each of the
8 cores handles 10 queries against the full replicated support set.

Per-core math (q=10 local queries, way=5, C=3840, 25 spatial positions,
levels d in {256,512,1024,2048}):
  1. subtract channel mean from spt/qry
  2. per level: corr2[(w,s),(q,t)] = sum_{c in l} S0[c,ws] Q0[c,qt]  (PE)
     GramS[(w,s),(w,s')] , GramQ[(q,t),(q,t')]                       (PE)
  3. l2-normalized corr = corr2 * rsqrt(diag GramS) * rsqrt(diag GramQ)
  4. gaussian-normalize + softmax over s (and t), sum over the other axis
     -> attn_s[q,(w,s)], attn_q[w,(q,t)]  (group reductions in free dim;
     cross-partition sums via indicator matmuls; transposes via PE)
  5. dot = attn_s^T corr2 attn_q ; |spt_pooled|^2 = attn_s^T GramS attn_s
     (w-block-diagonal), |qry_pooled|^2 = attn_q^T GramQ attn_q
     (q-block-diagonal); cosine similarity / 0.2.
"""

import json

import numpy as np
from concourse import bass, mybir
from concourse import bass2jax as _b2j
from concourse import bass_utils as _bu
from concourse.tile import TileContext
from concourse.bass_utils import run_bass_kernel_spmd


def _split_multiwaits(bir_json: bytes) -> bytes:
    """Walrus in this env allows one sync-wait per compute instruction.

    Split every multi-wait instruction: hoist all but the last wait onto
    fresh single-wait RegisterMove carriers (same engine, immediately
    preceding), cloned from the preamble zero-reg writes.
    """
    j = json.loads(bir_json)
    tmpl = {}
    for f in j["functions"]:
        for b in f["blocks"]:
            for i in b["instructions"]:
                if i["opcode"] == "RegisterMove":
                    for o in i.get("outs", []):
                        if str(o.get("regref", "")).endswith("_zero"):
                            tmpl.setdefault(i["engine"], i)
    uid = [0]

    def carrier(engine, wait):
        t = tmpl[engine]
        uid[0] += 1
        return {
            "debug": t.get("debug", 0),
            "engine": engine,
            "ins": [dict(x) for x in t["ins"]],
            "name": f"IW-{uid[0]}",
            "opcode": "RegisterMove",
            "outs": [dict(x) for x in t["outs"]],
            "sync_info": {"on_update": [], "on_wait": [wait]},
        }

    for f in j["functions"]:
        for b in f["blocks"]:
            out = []
            for i in b["instructions"]:
                si = i.get("sync_info")
                ow = si.get("on_wait") if si else None
                if ow and len(ow) > 1:
                    for w in ow[:-1]:
                        out.append(carrier(i["engine"], w))
                    si["on_wait"] = [ow[-1]]
                out.append(i)
            b["instructions"] = out
    return json.dumps(j).encode()


_orig_cbk = _bu.compile_bir_kernel


def _patched_cbk(bir_json, tmpdir, neff_name="file.neff"):
    return _orig_cbk(_split_multiwaits(bir_json), tmpdir, neff_name=neff_name)


for _mod in (_b2j, _bu):
    if getattr(_mod, "compile_bir_kernel", None) is _orig_cbk:
        _mod.compile_bir_kernel = _patched_cbk

F32 = mybir.dt.float32
AX = mybir.AxisListType.X
ADD = mybir.AluOpType.add
EXP = mybir.ActivationFunctionType.Exp
RSQRT = mybir.ActivationFunctionType.Rsqrt
SQRT = mybir.ActivationFunctionType.Sqrt
COPY = mybir.ActivationFunctionType.Copy

HYPER = [256, 512, 1024, 2048]
C = 3840
WAY = 5
SS = 25           # fs*fs
NQ = 75
NQL = 10          # queries per core
NCORES = 8
P = 128
NCH = C // P      # 30 channel chunks
WS = WAY * SS     # 125
QT = NQL * SS     # 250
T_ATTN = 5.0
L2_EPS = 1e-6
GN_EPS = 1e-5
# chunk ranges per level (channels /128)
LCH = []
off = 0
for d in HYPER:
    LCH.append((off // P, (off + d) // P))
    off += d

_CACHE = {}
LINEARIZE = False
DEBUG_NLEVELS = 4  # for sim ablation only; harness uses the default


def _build_nc():
    nc = bass.Bass()
    for val in (L2_EPS, GN_EPS):
        t = nc.alloc_sbuf_tensor(f"const-f32-{val}", [128, 1], F32)
        nc.gpsimd.memset(t.ap(), val)
        nc.const_aps.aps[(F32, val)] = t.ap()
    nc.all_engine_barrier()
    sT = nc.declare_dram_parameter("sT", [C, WS], F32, isOutput=False)
    qT = nc.declare_dram_parameter("qT", [C, QT], F32, isOutput=False)
    ident = nc.declare_dram_parameter("ident", [WS, WS], F32, isOutput=False)
    ones_col = nc.declare_dram_parameter("ones_col", [P, 1], F32, isOutput=False)
    ones_row = nc.declare_dram_parameter("ones_row", [1, P], F32, isOutput=False)
    w_ind = nc.declare_dram_parameter("w_ind", [WS, WAY], F32, isOutput=False)
    w_indT = nc.declare_dram_parameter("w_indT", [WAY, WS], F32, isOutput=False)
    q_ind = nc.declare_dram_parameter("q_ind", [WS, WAY], F32, isOutput=False)
    w_mask = nc.declare_dram_parameter("w_mask", [WS, WS], F32, isOutput=False)
    q_mask = nc.declare_dram_parameter("q_mask", [WS, WS], F32, isOutput=False)
    out_d = nc.declare_dram_parameter("out", [NQL, WAY], F32, isOutput=True)

    with TileContext(nc, linearize=LINEARIZE) as tc:
        with (
            tc.tile_pool(name="const", bufs=1) as const,
            tc.tile_pool(name="qdat", bufs=NCH) as qdat,
            tc.tile_pool(name="sdat", bufs=NCH) as sdat,
            tc.tile_pool(name="work", bufs=3) as work,
            tc.tile_pool(name="small", bufs=3) as small,
            tc.tile_pool(name="accum", bufs=1) as accum,
            tc.tile_pool(name="ps_corr", bufs=1, space="PSUM") as ps_corr,
            tc.tile_pool(name="ps_gram", bufs=1, space="PSUM") as ps_gram,
            tc.tile_pool(name="ps_sm", bufs=3, space="PSUM") as ps_sm,
        ):
            # ---- constants ----
            I125 = const.tile([WS, WS], F32)
            nc.sync.dma_start(out=I125[:, :], in_=ident[:, :])
            ONES = const.tile([P, 1], F32)
            nc.sync.dma_start(out=ONES[:, :], in_=ones_col[:, :])
            ONESR = const.tile([1, P], F32)
            nc.sync.dma_start(out=ONESR[:, :], in_=ones_row[:, :])
            WIND = const.tile([WS, WAY], F32)
            nc.sync.dma_start(out=WIND[:, :], in_=w_ind[:, :])
            WINDT = const.tile([WAY, WS], F32)
            nc.sync.dma_start(out=WINDT[:, :], in_=w_indT[:, :])
            QIND = const.tile([WS, WAY], F32)
            nc.sync.dma_start(out=QIND[:, :], in_=q_ind[:, :])
            WMASK = const.tile([WS, WS], F32)
            nc.sync.dma_start(out=WMASK[:, :], in_=w_mask[:, :])
            QMASK = const.tile([WS, WS], F32)
            nc.sync.dma_start(out=QMASK[:, :], in_=q_mask[:, :])

            # ---- load data + channel sums (serial DVE accumulation) ----
            qch = []
            sch = []
            for k in range(NCH):
                qk = qdat.tile([P, QT], F32, tag="qk")
                nc.sync.dma_start(out=qk[:, :], in_=qT[k * P:(k + 1) * P, :])
                sk = sdat.tile([P, WS], F32, tag="sk")
                nc.sync.dma_start(out=sk[:, :], in_=sT[k * P:(k + 1) * P, :])
                qch.append(qk)
                sch.append(sk)

            def _par_sum(tiles, width, tag, nacc=4):
                accs = []
                for a in range(nacc):
                    sub = tiles[a::nacc]
                    acc = work.tile([P, width], F32, tag=f"{tag}{a}",
                                    name=f"{tag}{a}", bufs=1)
                    nc.vector.tensor_add(acc[:, :], sub[0][:, :],
                                         sub[1][:, :])
                    for t in sub[2:]:
                        nc.vector.tensor_add(acc[:, :], acc[:, :], t[:, :])
                    accs.append(acc)
                while len(accs) > 1:
                    nc.vector.tensor_add(accs[0][:, :], accs[0][:, :],
                                         accs[1][:, :])
                    accs = [accs[0]] + accs[2:]
                return accs[0]

            qsum = _par_sum(qch, QT, "tq")
            ssum = _par_sum(sch, WS, "ts")
            ps_mq = ps_sm.tile([1, QT], F32, tag="ps")
            nc.tensor.matmul(ps_mq[:, :], ONES[:, :], qsum[:, :],
                             start=True, stop=True)
            ps_ms = ps_sm.tile([1, WS], F32, tag="ps")
            nc.tensor.matmul(ps_ms[:, :], ONES[:, :], ssum[:, :],
                             start=True, stop=True)
            mq_row = small.tile([1, QT], F32, tag="mqrow")
            nc.scalar.activation(mq_row[:, :], ps_mq[:, :], COPY, scale=1.0 / C)
            ms_row = small.tile([1, WS], F32, tag="msrow")
            nc.scalar.activation(ms_row[:, :], ps_ms[:, :], COPY, scale=1.0 / C)
            pmqb = ps_sm.tile([P, QT], F32, tag="ps")
            nc.tensor.matmul(pmqb[:, :], ONESR[:, :], mq_row[:, :],
                             start=True, stop=True)
            mq_bc = work.tile([P, QT], F32, tag="mqbc")
            nc.vector.tensor_copy(mq_bc[:, :], pmqb[:, :])
            pmsb = ps_sm.tile([P, WS], F32, tag="ps")
            nc.tensor.matmul(pmsb[:, :], ONESR[:, :], ms_row[:, :],
                             start=True, stop=True)
            ms_bc = work.tile([P, WS], F32, tag="msbc")
            nc.vector.tensor_copy(ms_bc[:, :], pmsb[:, :])

            # ---- subtract means (in place; split across DVE/GpSimd) ----
            for k in range(NCH):
                nc.vector.tensor_sub(qch[k][:, :], qch[k][:, :], mq_bc[:, :])
                nc.gpsimd.tensor_sub(sch[k][:, :], sch[k][:, :], ms_bc[:, :])

            # accumulators [5, WAY] per query-half (avoid partition offsets)
            dot_acc = [accum.tile([WAY, WAY], F32, tag=f"dot{h}", name=f"dot{h}")
                       for h in range(2)]
            s2_acc = [accum.tile([WAY, WAY], F32, tag=f"s2a{h}", name=f"s2a{h}")
                      for h in range(2)]
            q2_acc = [accum.tile([WAY, WAY], F32, tag=f"q2a{h}", name=f"q2a{h}")
                      for h in range(2)]

            for li, (k0, k1) in enumerate(LCH[:DEBUG_NLEVELS]):
                # ---- big matmuls for this level ----
                pcorr0 = ps_corr.tile([WS, WS], F32, tag="pcorr0")
                pcorr1 = ps_corr.tile([WS, WS], F32, tag="pcorr1")
                pgs = ps_gram.tile([WS, WS], F32, tag="pgs")
                pgq0 = ps_gram.tile([WS, WS], F32, tag="pgq0")
                pgq1 = ps_gram.tile([WS, WS], F32, tag="pgq1")
                for k in range(k0, k1):
                    st, sp = (k == k0), (k == k1 - 1)
                    nc.tensor.matmul(pcorr0[:, :], sch[k][:, :],
                                     qch[k][:, 0:WS], start=st, stop=sp)
                    nc.tensor.matmul(pcorr1[:, :], sch[k][:, :],
                                     qch[k][:, WS:QT], start=st, stop=sp)
                    nc.tensor.matmul(pgs[:, :], sch[k][:, :], sch[k][:, :],
                                     start=st, stop=sp)
                    nc.tensor.matmul(pgq0[:, :], qch[k][:, 0:WS],
                                     qch[k][:, 0:WS], start=st, stop=sp)
                    nc.tensor.matmul(pgq1[:, :], qch[k][:, WS:QT],
                                     qch[k][:, WS:QT], start=st, stop=sp)

                # evict raw corr + grams to SBUF
                c2 = work.tile([WS, QT], F32, tag="c2")
                nc.scalar.activation(c2[:, 0:WS], pcorr0[:, :], COPY)
                nc.scalar.activation(c2[:, WS:QT], pcorr1[:, :], COPY)
                gs = work.tile([WS, WS], F32, tag="gs")
                nc.vector.tensor_copy(gs[:, :], pgs[:, :])
                gq0 = work.tile([WS, WS], F32, tag="gq0")
                nc.vector.tensor_copy(gq0[:, :], pgq0[:, :])
                gq1 = work.tile([WS, WS], F32, tag="gq1")
                nc.vector.tensor_copy(gq1[:, :], pgq1[:, :])

                # ---- norms from gram diagonals ----
                # support: diag as [WS,1] column (per-partition scalar)
                gsd = small.tile([WS, WS], F32, tag="gsd")
                nc.vector.tensor_mul(gsd[:, :], gs[:, :], I125[:, :])
                ns_col = small.tile([WS, 1], F32, tag="nscol")
                nc.vector.tensor_reduce(ns_col[:, :], gsd[:, :], AX, ADD)
                inv_ns = small.tile([WS, 1], F32, tag="invns")
                nc.scalar.activation(inv_ns[:, :], ns_col[:, :], SQRT,
                                     bias=L2_EPS)
                nc.vector.reciprocal(inv_ns[:, :], inv_ns[:, :])
                # query: diag of each gram half -> row [1,125] via ones-matmul
                inv_nq_row = small.tile([1, QT], F32, tag="invnq")
                for h, gq in enumerate((gq0, gq1)):
                    gqd = small.tile([WS, WS], F32, tag="gqd")
                    nc.vector.tensor_mul(gqd[:, :], gq[:, :], I125[:, :])
                    pdg = ps_sm.tile([1, WS], F32, tag="ps")
                    nc.tensor.matmul(pdg[:, :], ONES[0:WS, :], gqd[:, :],
                                     start=True, stop=True)
                    nc.scalar.activation(
                        inv_nq_row[:, h * WS:(h + 1) * WS], pdg[:, :], SQRT,
                        bias=L2_EPS)
                nc.vector.reciprocal(inv_nq_row[:, :], inv_nq_row[:, :])
                inv_nq_bc = ps_sm.tile([WS, QT], F32, tag="ps")
                nc.tensor.matmul(inv_nq_bc[:, :], ONESR[:, 0:WS],
                                 inv_nq_row[:, :], start=True, stop=True)

                # normalized corr
                cn = work.tile([WS, QT], F32, tag="cn")
                nc.vector.tensor_scalar_mul(cn[:, :], c2[:, :], inv_ns[:, 0:1])
                nc.vector.tensor_mul(cn[:, :], cn[:, :], inv_nq_bc[:, :])

                # ---- attn_q: gn+softmax over t (free groups of 25) ----
                softq = work.tile([WS, QT], F32, tag="softq")
                _gn_softmax_groups(nc, small, cn, softq, WS, NQL)
                # sum over s within w: [5, QT]
                paq = ps_sm.tile([WAY, QT], F32, tag="ps")
                nc.tensor.matmul(paq[:, :], WIND[:, :], softq[:, :],
                                 start=True, stop=True)
                attn_q = small.tile([WAY, QT], F32, tag="attnq")
                nc.vector.tensor_copy(attn_q[:, :], paq[:, :])

                # ---- attn_s halves: transpose, gn+softmax over s ----
                # both halves concatenated in free dim -> one wide gn pass
                cnT = work.tile([WS, 2 * WS], F32, tag="cnT")
                for h in range(2):
                    pT = ps_sm.tile([WS, WS], F32, tag="ps")
                    nc.tensor.transpose(pT[:, :],
                                        cn[:, h * WS:(h + 1) * WS],
                                        I125[:, :])
                    nc.vector.tensor_copy(cnT[:, h * WS:(h + 1) * WS],
                                          pT[:, :])
                softs = work.tile([WS, 2 * WS], F32, tag="softs")
                _gn_softmax_groups(nc, small, cnT, softs, WS, 2 * WAY)
                attn_sh = []
                for h in range(2):
                    pas = ps_sm.tile([WAY, WS], F32, tag="ps")
                    nc.tensor.matmul(pas[:, :], QIND[:, :],
                                     softs[:, h * WS:(h + 1) * WS],
                                     start=True, stop=True)
                    a_s = small.tile([WAY, WS], F32, tag=f"attns{h}")
                    nc.vector.tensor_copy(a_s[:, :], pas[:, :])
                    attn_sh.append(a_s)

                # ---- dot = attn_s^T c2 attn_q ----
                pqbc = ps_sm.tile([WS, QT], F32, tag="ps")
                nc.tensor.matmul(pqbc[:, :], WINDT[:, :], attn_q[:, :],
                                 start=True, stop=True)
                u = work.tile([WS, QT], F32, tag="u")
                nc.vector.tensor_mul(u[:, :], c2[:, :], pqbc[:, :])
                v = small.tile([WS, NQL], F32, tag="v")
                nc.vector.tensor_reduce(
                    v[:, :], u[:, :].rearrange("p (q t) -> p q t", t=SS),
                    AX, ADD)
                gsm = work.tile([WS, WS], F32, tag="gsm")
                nc.vector.tensor_mul(gsm[:, :], gs[:, :], WMASK[:, :])
                for h in range(2):
                    pvT = ps_sm.tile([WAY, WS], F32, tag="ps")
                    nc.tensor.matmul(pvT[:, :],
                                     v[:, h * WAY:(h + 1) * WAY],
                                     I125[:, :], start=True, stop=True)
                    pl = small.tile([WAY, WS], F32, tag="pl")
                    nc.vector.tensor_mul(pl[:, :], attn_sh[h][:, :],
                                         pvT[:, :])
                    dl = small.tile([WAY, WAY], F32, tag="dl")
                    nc.vector.tensor_reduce(
                        dl[:, :],
                        pl[:, :].rearrange("p (w s) -> p w s", s=SS),
                        AX, ADD)
                    if li == 0:
                        nc.vector.tensor_copy(dot_acc[h][:, :], dl[:, :])
                    else:
                        nc.vector.tensor_add(dot_acc[h][:, :],
                                             dot_acc[h][:, :], dl[:, :])

                    # ---- s2 = attn_s^T (GramS . wmask) attn_s ----
                    psT = ps_sm.tile([WS, WAY], F32, tag="ps")
                    nc.tensor.matmul(psT[:, :], attn_sh[h][:, :],
                                     I125[0:WAY, 0:WAY],
                                     start=True, stop=True)
                    asT = small.tile([WS, WAY], F32, tag="asT")
                    nc.vector.tensor_copy(asT[:, :], psT[:, :])
                    py = ps_sm.tile([WS, WAY], F32, tag="ps")
                    nc.tensor.matmul(py[:, :], gsm[:, :], asT[:, :],
                                     start=True, stop=True)
                    z = small.tile([WS, WAY], F32, tag="z")
                    nc.vector.tensor_mul(z[:, :], asT[:, :], py[:, :])
                    pwq = ps_sm.tile([WAY, WAY], F32, tag="ps")
                    nc.tensor.matmul(pwq[:, :], WIND[:, :], z[:, :],
                                     start=True, stop=True)
                    s2wq = small.tile([WAY, WAY], F32, tag="s2wq")
                    nc.vector.tensor_copy(s2wq[:, :], pwq[:, :])
                    ps2 = ps_sm.tile([WAY, WAY], F32, tag="ps")
                    nc.tensor.matmul(ps2[:, :], s2wq[:, :],
                                     I125[0:WAY, 0:WAY],
                                     start=True, stop=True)
                    if li == 0:
                        nc.vector.tensor_copy(s2_acc[h][:, :], ps2[:, :])
                    else:
                        nc.vector.tensor_add(s2_acc[h][:, :],
                                             s2_acc[h][:, :], ps2[:, :])

                # ---- q2 = attn_q^T (GramQ . qmask) attn_q (per half) ----
                for h, gq in enumerate((gq0, gq1)):
                    gqm = work.tile([WS, WS], F32, tag="gqm")
                    nc.vector.tensor_mul(gqm[:, :], gq[:, :], QMASK[:, :])
                    pqT = ps_sm.tile([WS, WAY], F32, tag="ps")
                    nc.tensor.matmul(pqT[:, :],
                                     attn_q[:, h * WS:(h + 1) * WS],
                                     I125[0:WAY, 0:WAY], start=True, stop=True)
                    aqT = small.tile([WS, WAY], F32, tag="aqT")
                    nc.vector.tensor_copy(aqT[:, :], pqT[:, :])
                    pz = ps_sm.tile([WS, WAY], F32, tag="ps")
                    nc.tensor.matmul(pz[:, :], gqm[:, :], aqT[:, :],
                                     start=True, stop=True)
                    zz = small.tile([WS, WAY], F32, tag="zz")
                    nc.vector.tensor_mul(zz[:, :], aqT[:, :], pz[:, :])
                    pq2 = ps_sm.tile([WAY, WAY], F32, tag="ps")
                    nc.tensor.matmul(pq2[:, :], QIND[:, :], zz[:, :],
                                     start=True, stop=True)
                    if li == 0:
                        nc.vector.tensor_copy(q2_acc[h][:, :], pq2[:, :])
                    else:
                        nc.vector.tensor_add(q2_acc[h][:, :],
                                             q2_acc[h][:, :], pq2[:, :])

            # ---- final cosine similarity (per query-half) ----
            for h in range(2):
                ns_f = small.tile([WAY, WAY], F32, tag="nsf")
                nc.scalar.activation(ns_f[:, :], s2_acc[h][:, :], SQRT)
                nq_f = small.tile([WAY, WAY], F32, tag="nqf")
                nc.scalar.activation(nq_f[:, :], q2_acc[h][:, :], SQRT)
                nc.vector.tensor_scalar_max(ns_f[:, :], ns_f[:, :], SS * 1e-8)
                nc.vector.tensor_scalar_max(nq_f[:, :], nq_f[:, :], SS * 1e-8)
                den = small.tile([WAY, WAY], F32, tag="den")
                nc.vector.tensor_mul(den[:, :], ns_f[:, :], nq_f[:, :])
                nc.vector.reciprocal(den[:, :], den[:, :])
                sim = small.tile([WAY, WAY], F32, tag="sim")
                nc.vector.tensor_mul(sim[:, :], dot_acc[h][:, :], den[:, :])
                nc.vector.tensor_scalar_mul(sim[:, :], sim[:, :], 1.0 / 0.2)
                nc.sync.dma_start(out=out_d[h * WAY:(h + 1) * WAY, :],
                                  in_=sim[:, :])
    return nc


def _gn_softmax_groups(nc, small, x, out, parts, ngroups):
    """out = softmax over groups of 25 (free axis) of gaussian-normalized x.

    x: [parts, ngroups*25]; gaussian normalize (unbiased var) within each
    group, divide by T_ATTN, exp, normalize to sum 1 within group.
    """
    g = ngroups
    x3 = x[:, :].rearrange("p (g t) -> p g t", t=SS)
    s1 = small.tile([parts, g], F32, tag="gn_s1")
    nc.vector.tensor_reduce(s1[:, :], x3, AX, ADD)
    xsq = small.tile([parts, g * SS], F32, tag="gn_xsq")
    nc.vector.tensor_mul(xsq[:, :], x[:, :], x[:, :])
    s2 = small.tile([parts, g], F32, tag="gn_s2")
    nc.vector.tensor_reduce(
        s2[:, :], xsq[:, :].rearrange("p (g t) -> p g t", t=SS), AX, ADD)
    # var = (s2 - s1^2/25)/24 ; invstd_sc = rsqrt(var+eps)/T_ATTN
    m2 = small.tile([parts, g], F32, tag="gn_m2")
    nc.vector.tensor_mul(m2[:, :], s1[:, :], s1[:, :])
    var = small.tile([parts, g], F32, tag="gn_var")
    nc.vector.tensor_scalar_mul(var[:, :], m2[:, :], 1.0 / (SS * (SS - 1)))
    v2 = small.tile([parts, g], F32, tag="gn_v2")
    nc.vector.tensor_scalar_mul(v2[:, :], s2[:, :], 1.0 / (SS - 1))
    nc.vector.tensor_sub(var[:, :], v2[:, :], var[:, :])
    inv = small.tile([parts, g], F32, tag="gn_inv")
    nc.scalar.activation(inv[:, :], var[:, :], SQRT, bias=GN_EPS)
    nc.vector.reciprocal(inv[:, :], inv[:, :])
    nc.vector.tensor_scalar_mul(inv[:, :], inv[:, :], 1.0 / T_ATTN)
    mean = small.tile([parts, g], F32, tag="gn_mean")
    nc.vector.tensor_scalar_mul(mean[:, :], s1[:, :], 1.0 / SS)
    # xhat = (x - mean)*invstd_sc ; e = exp(xhat)
    out3 = out[:, :].rearrange("p (g t) -> p g t", t=SS)
    nc.vector.tensor_sub(
        out3, x3, mean[:, :].unsqueeze(2).to_broadcast([parts, g, SS]))
    nc.vector.tensor_mul(
        out3, out3, inv[:, :].unsqueeze(2).to_broadcast([parts, g, SS]))
    nc.scalar.activation(out[:, :], out[:, :], EXP)
    den = small.tile([parts, g], F32, tag="gn_den")
    nc.vector.tensor_reduce(den[:, :], out3, AX, ADD)
    nc.vector.reciprocal(den[:, :], den[:, :])
    nc.vector.tensor_mul(
        out3, out3, den[:, :].unsqueeze(2).to_broadcast([parts, g, SS]))


def _constants():
    ident = np.eye(WS, dtype=np.float32)
    ones_col = np.ones((P, 1), dtype=np.float32)
    w_ind = np.zeros((WS, WAY), dtype=np.float32)
    for w in range(WAY):
        w_ind[w * SS:(w + 1) * SS, w] = 1.0
    q_ind = np.zeros((WS, WAY), dtype=np.float32)
    for q in range(WAY):
        q_ind[q * SS:(q + 1) * SS, q] = 1.0
    w_mask = np.kron(np.eye(WAY, dtype=np.float32),
                     np.ones((SS, SS), dtype=np.float32))
    return {
        "ident": ident, "ones_col": ones_col,
        "ones_row": np.ones((1, P), dtype=np.float32), "w_ind": w_ind,
        "w_indT": np.ascontiguousarray(w_ind.T), "q_ind": q_ind,
        "w_mask": w_mask, "q_mask": w_mask.copy(),
    }


def kernel(spt: np.ndarray, qry: np.ndarray) -> np.ndarray:
    if "nc" not in _CACHE:
        _CACHE["nc"] = _build_nc()
        _CACHE["consts"] = _constants()
    nc = _CACHE["nc"]
    consts = _CACHE["consts"]

    s = np.asarray(spt, dtype=np.float32).reshape(WAY, C, SS)
    sT = np.ascontiguousarray(s.transpose(1, 0, 2).reshape(C, WS))
    q = np.asarray(qry, dtype=np.float32).reshape(NQ, C, SS)
    qpad = np.zeros((NCORES * NQL, C, SS), dtype=np.float32)
    qpad[:NQ] = q

    in_maps = []
    for core in range(NCORES):
        qc = qpad[core * NQL:(core + 1) * NQL]          # [10, C, 25]
        qTc = np.ascontiguousarray(qc.transpose(1, 0, 2).reshape(C, QT))
        m = {"sT": sT, "qT": qTc}
        m.update(consts)
        in_maps.append(m)

    res = run_bass_kernel_spmd(nc, in_maps, list(range(NCORES)))
    out = np.concatenate([res.results[i]["out"] for i in range(NCORES)],
                         axis=0)
    return np.ascontiguousarray(out[:NQ])

